# revision 57
# baseline (speedup 1.0000x reference)
"""Multi-head causal attention block (qkv -> softmax(QK^T/sqrt(d)+mask) V -> proj)
on 8 Trainium2 NeuronCores.

Sharding: 8 cores = 4 batches (data parallel) x 2 head-groups of 8 heads
(tensor parallel: W_qkv column-sharded, W_proj row-sharded). Each core
computes a partial projection output for its (batch, head-group); the host
sums the two partials per batch (the "all-reduce") and adds b_proj.

Core kernel (per core, all matmuls fp16 with fp32 psum accumulate):
  - qT/kT computed in [d, n] layout, v in [n, d] layout (x pre-transposed on
    host so every matmul contracts over the partition dim).
  - attention uses transposed scores S^T[k, q] = (kT_tile).T @ qT so that the
    softmax denominator comes for free from a ones-column augmented V
    (out[64] = column sums) and P^T never needs an on-chip transpose.
  - causal structure: fully-masked 128x128 blocks are skipped; on diagonal
    blocks the mask is applied as a post-exp multiply by host-precomputed
    exp(mask) (exp(s+m) = exp(s)*exp(m)), avoiding any PSUM read-modify-write.
  - exp on ScalarE without max subtraction (logits are O(5) here; exact for
    the softmax up to fp rounding).

Scheduling (the perf-critical part): the kernel is one software pipeline.
qkv/proj work is emitted as small "filler" units interleaved between
attention score/PV entries so the PE never idles while ScalarE works
through the softmax exps.  Filler placement is load-balanced against the
causal triangle: group i4's windows carry 4*(i4+1) exps (~1.2us each on
ACT) but only ~3.7*(i4+1)us of score/PV PE work, so early groups get the
qkv fillers and ALL proj units are deferred to the last two groups (their
attn tiles stay alive via aop bufs=4).  Engine placement: qkv-unit and proj(0) psum casts go to ACT (which has
exp slack in every group that carries them, i4=0..2) so DVE stays free
for the mask/normalize ops that gate the PV chains; i4=3's proj casts
stay on DVE (ACT is saturated there).  Out-DMAs are dispatched from the
idle Sync queue (DMA dispatches execute on the issuing engine's queue,
~0.7us each).
PSUM: 3 score/unit tiles (6 banks) + 2 PV accumulators - score matmuls
ride out ~3 entries of exp jitter; the PV window handoff is covered by
emitting the Pool broadcast at window end and deferring only the final
DVE multiply.  Startup is DMA-fabric-bound (~240GB/s aggregate): inputs
are fetched as 512-col pieces, demand-ordered (q/x pairs kt-major, k
promoted over late q, v last, the 1MB proj weights deferred until after
qkv0 emission) and interleaved over the two hardware DGE queues.
Scores are packed [2w] tight in PSUM so exp never covers dead columns;
the two heads of a pair run concurrently via PE row-group array packing.
"""

import numpy as np

B, N, C = 4, 2048, 1024
H, D = 16, 64
G = 2                  # head groups (cores = B * G = 8)
HPC = H // G           # heads per core
DG = HPC * D           # 512 = per-core qkv width per projection
NT = N // 128          # 16 k/n tiles
QG = N // 512          # 4 q groups
VW = 65                # v_aug width per head (ones col + 64 dims)

_CACHE = {}


def _classify_blocks(attn_mask):
    """Per 128x128 block (j=k-tile, i=q-tile): 0 all-zero, 1 all-masked, 2 mixed."""
    sub = np.empty((NT, NT), dtype=np.int8)
    for j in range(NT):
        for i in range(NT):
            blk = attn_mask[i * 128:(i + 1) * 128, j * 128:(j + 1) * 128]
            if np.all(blk == 0.0):
                sub[j, i] = 0
            elif np.all(blk <= -150.0):
                sub[j, i] = 1
            else:
                sub[j, i] = 2
    return sub


def _build_plan(attn_mask):
    """Plan: for each (qgroup i4, k-tile j) either skip or compute cols
    [lo,hi) (128-units within the 512-wide group) with optional mask add
    (segment id, add_lo, add_hi). Returns plan + concatenated mask segments."""
    sub = _classify_blocks(attn_mask)
    segs = {}
    seg_list = []
    plan = []  # list over i4 of list of (j, lo, hi, mseg or None)
    for i4 in range(QG):
        entries = []
        for j in range(NT):
            states = [sub[j, 4 * i4 + qc] for qc in range(4)]
            keep = [qc for qc in range(4) if states[qc] != 1]
            if not keep:
                continue
            lo, hi = min(keep), max(keep) + 1
            need = [qc for qc in range(lo, hi) if states[qc] != 0]
            mseg = None
            if need:
                alo, ahi = min(need), max(need) + 1
                i0 = (4 * i4 + alo) * 128
                i1 = (4 * i4 + ahi) * 128
                seg = np.exp(np.ascontiguousarray(
                    attn_mask[i0:i1, j * 128:(j + 1) * 128].T).astype(
                        np.float64)).astype(np.float32)
                key = (ahi - alo, seg.tobytes())
                if key not in segs:
                    segs[key] = sum(s.shape[1] // 128 for s in seg_list)
                    seg_list.append(seg)
                mseg = (segs[key], alo, ahi)
            entries.append((j, lo, hi, mseg))
        plan.append(entries)
    if seg_list:
        masks_np = np.concatenate(seg_list, axis=1)
    else:
        masks_np = np.zeros((128, 128), dtype=np.float32)
    return plan, masks_np


def _build_program(plan, mask_width):
    import concourse.mybir as mybir
    import concourse.tile as tile
    from concourse import bacc

    F32 = mybir.dt.float32
    F16 = mybir.dt.float16
    AF = mybir.ActivationFunctionType

    nc = bacc.Bacc("TRN2", target_bir_lowering=False, debug=False, num_devices=8)
    xT = nc.dram_tensor("xT", [C, N], F16, kind="ExternalInput").ap()
    wqkv = nc.dram_tensor("wqkv", [C, 3 * DG], F16, kind="ExternalInput").ap()
    wp = nc.dram_tensor("wp", [DG, C], F16, kind="ExternalInput").ap()
    masks = nc.dram_tensor("masks", [128, mask_width], F16, kind="ExternalInput").ap()
    ones = nc.dram_tensor("ones", [128, 128], F16, kind="ExternalInput").ap()
    out = nc.dram_tensor("out", [N, C], F16, kind="ExternalOutput").ap()

    with tile.TileContext(nc) as tc:
        with (tc.tile_pool(name="pers", bufs=1) as pers,
              tc.tile_pool(name="mmps", bufs=3, space="PSUM") as psA,
              tc.tile_pool(name="pvps", bufs=2, space="PSUM") as psB,
              tc.tile_pool(name="xp", bufs=6) as xp,
              tc.tile_pool(name="wqp", bufs=1) as wqp,
              tc.tile_pool(name="ep", bufs=6) as ep,
              tc.tile_pool(name="aop", bufs=4) as aop,
              tc.tile_pool(name="nrm", bufs=3) as nrm,
              tc.tile_pool(name="wpp", bufs=1) as wpp,
              tc.tile_pool(name="op", bufs=3) as op):
            sq = pers.tile([128, 4 * N], F16, tag="sq")
            sk = pers.tile([128, 4 * N], F16, tag="sk")
            sv = pers.tile([128, NT * HPC * VW], F16, tag="sv")
            smask = pers.tile([128, mask_width], F16, tag="smask")
            sones = pers.tile([128, 128], F16, tag="sones")
            # weights in 4 separate tiles so early matmuls only wait on
            # their own chunk's DMA, not the full weight load
            swqc = [wqp.tile([128, 2 * 1536], F16, tag=f"w{c}", name=f"swq{c}")
                    for c in range(4)]
            swp = wpp.tile([128, 4 * C], F16, tag="wp")

            def swq(kt, off, width):
                base = (kt % 2) * 1536 + off
                return swqc[kt // 2][:, base:base + width]

            xtiles = {}

            def fetch_x(ng, eng=None, split=1):
                eng = eng or nc.sync
                xh = []
                for half in range(2):
                    xt = xp.tile([128, 4 * 512], F16, tag="x",
                                 name=f"xt{ng}_{half}")
                    for q in range(split):
                        kn = 4 // split
                        eng.dma_start(
                            xt[:].rearrange("p (kt n) -> p kt n", kt=4)
                              [:, q * kn:(q + 1) * kn, :],
                            xT.rearrange("(kt p) n -> p kt n", p=128)
                              [:, half * 4 + q * kn:half * 4 + (q + 1) * kn,
                               ng * 512:(ng + 1) * 512])
                    xh.append(xt)
                xtiles[ng] = xh

            # ---- startup DMAs: 512-col pieces, demand-ordered and
            # interleaved over the two hardware DGE queues (sync/scalar,
            # ~137GB/s each; a single dma_start tops out at ~120 GB/s).
            # emit_qkv0 is kt-major and consumes (w_q[kt], x0[kt]) pairs at
            # ~1.04us per kt, so supply pieces in exactly that order. ----
            def fetch_w_piece(kt, proj, eng):
                base = (kt % 2) * 1536 + proj * 512
                eng.dma_start(
                    swqc[kt // 2][:, base:base + 512],
                    wqkv.rearrange("(kt p) c -> p kt c", p=128)
                        [:, kt, proj * 512:proj * 512 + 512])

            def fetch_x_piece(kt, eng):
                xt = xtiles[0][kt // 4]
                eng.dma_start(
                    xt[:].rearrange("p (k n) -> p k n", k=4)
                      [:, kt % 4:kt % 4 + 1, :],
                    xT.rearrange("(k p) n -> p k n", p=128)
                      [:, kt:kt + 1, 0:512])

            xtiles[0] = [xp.tile([128, 4 * 512], F16, tag="x",
                                 name=f"xt0_{half}") for half in range(2)]
            # demand-ordered piece list: q/x pairs for kt 0..3, then k 0..3
            # promoted ahead of q 4..7 (k consumption starts at ~15us while
            # the q/x surplus has built up), then the rest, v last
            pieces = []
            for kt in range(4):
                pieces += [('w', kt, 0), ('x', kt, None)]
            pieces += [('w', kt, 1) for kt in range(4)]
            for kt in range(4, 8):
                pieces += [('w', kt, 0), ('x', kt, None)]
            pieces += [('w', kt, 1) for kt in range(4, 8)]
            pieces += [('w', kt, 2) for kt in range(8)]
            for idx, (kind, kt, proj) in enumerate(pieces):
                # all three DMA queues: two queues measured only ~118GB/s
                # aggregate here while three pulled ~233GB/s (in-flight
                # transfers fair-share the 16 DMA engines, so more queue
                # slots = more fabric)
                eng = (nc.sync, nc.gpsimd, nc.scalar)[idx % 3]
                if kind == 'w':
                    fetch_w_piece(kt, proj, eng)
                else:
                    fetch_x_piece(kt, eng)
            nc.gpsimd.dma_start(sones[:], ones)
            nc.gpsimd.dma_start(smask[:], masks)
            # ones column (at index 64) for every (n-tile, head)
            nc.vector.tensor_copy(
                sv[:].rearrange("p (t c) -> p t c", c=VW)[:, :, 64:65],
                sones[:])

            def xslice(ng, kt, a, b):
                return xtiles[ng][kt // 4][
                    :, (kt % 4) * 512 + a:(kt % 4) * 512 + b]

            # ---------------- QKV projection units ----------------
            # One unit = one [128,1024] psum tile = 2 accumulation groups
            # (16 matmuls) + one psum->sbuf copy. ~3.7us of PE each.
            # copy_eng: the psum->sbuf cast engine.  DVE casts sit on the
            # PE-critical psA rotation, so units emitted in EARLY attention
            # groups (where ACT has exp slack) cast on ACT instead.
            def make_qkv_units(ng, copy_eng=None):
                units = []

                def do_copy(dview, sview):
                    if copy_eng is nc.scalar:
                        nc.scalar.copy(dview, sview)
                    else:
                        (copy_eng or nc.vector).tensor_copy(dview, sview)

                def qk_unit(proj, mp):
                    def emit():
                        ps = psA.tile([128, 1024], F32, tag="mm2")
                        for sub in range(2):
                            mt = mp * 2 + sub
                            for kt in range(8):
                                nc.tensor.matmul(
                                    ps[:, sub * 512:sub * 512 + 512],
                                    swq(kt, proj * DG + mt * 128, 128),
                                    xslice(ng, kt, 0, 512),
                                    start=(kt == 0), stop=(kt == 7))
                        dst = sq if proj == 0 else sk
                        dview = (dst[:].rearrange("p (mt n) -> p mt n", n=N)
                                 [:, mp * 2:mp * 2 + 2,
                                  ng * 512:ng * 512 + 512])
                        sview = ps[:].rearrange("p (s n) -> p s n", n=512)
                        do_copy(dview, sview)
                    return emit

                def v_unit(sp):
                    def emit():
                        ps = psA.tile([128, 1024], F32, tag="mm2")
                        for sub in range(2):
                            for kt in range(8):
                                nc.tensor.matmul(
                                    ps[:, sub * 512:sub * 512 + 512],
                                    xslice(ng, kt, (sp * 2 + sub) * 128,
                                           (sp * 2 + sub) * 128 + 128),
                                    swq(kt, 1024, 512),
                                    start=(kt == 0), stop=(kt == 7))
                        nt_i = ng * 4 + sp * 2
                        dview = (sv[:].rearrange("p (t h c) -> p t h c",
                                                 h=HPC, c=VW)
                                 [:, nt_i:nt_i + 2, :, 0:D])
                        sview = ps[:].rearrange("p (s h c) -> p s h c",
                                                s=2, c=D)
                        do_copy(dview, sview)
                    return emit

                for proj in range(2):
                    for mp in range(2):
                        units.append(qk_unit(proj, mp))
                for sp in range(2):
                    units.append(v_unit(sp))
                return units

            # qkv(0) with the q (and k) pairs kt-major so the PE starts on
            # weight chunk 0 instead of waiting for the full weight DMA.
            def emit_qkv0(proj):
                tiles = [psA.tile([128, 1024], F32, tag="mm2",
                                  name=f"q0_{proj}_{mp}") for mp in range(2)]
                for kt in range(8):
                    for mp in range(2):
                        for sub in range(2):
                            mt = mp * 2 + sub
                            nc.tensor.matmul(
                                tiles[mp][:, sub * 512:sub * 512 + 512],
                                swq(kt, proj * DG + mt * 128, 128),
                                xslice(0, kt, 0, 512),
                                start=(kt == 0), stop=(kt == 7))
                dst = sq if proj == 0 else sk
                for mp in range(2):
                    nc.vector.tensor_copy(
                        dst[:].rearrange("p (mt n) -> p mt n", n=N)
                           [:, mp * 2:mp * 2 + 2, 0:512],
                        tiles[mp][:].rearrange("p (s n) -> p s n", n=512))

            emit_qkv0(0)
            emit_qkv0(1)
            # v pair, also kt-major
            vtiles = [psA.tile([128, 1024], F32, tag="mm2",
                               name=f"v0_{sp}") for sp in range(2)]
            for kt in range(8):
                for sp in range(2):
                    for sub in range(2):
                        nc.tensor.matmul(
                            vtiles[sp][:, sub * 512:sub * 512 + 512],
                            xslice(0, kt, (sp * 2 + sub) * 128,
                                   (sp * 2 + sub) * 128 + 128),
                            swq(kt, 1024, 512),
                            start=(kt == 0), stop=(kt == 7))
            for sp in range(2):
                nc.vector.tensor_copy(
                    sv[:].rearrange("p (t h c) -> p t h c", h=HPC, c=VW)
                      [:, sp * 2:sp * 2 + 2, :, 0:D],
                    vtiles[sp][:].rearrange("p (s h c) -> p s h c", s=2, c=D))
            # proj weights (1MB) aren't consumed until i4=2; fetching them
            # here keeps the startup DMA fabric (~240GB/s aggregate) free
            # for the qkv0 weight/x pieces the PE is actually waiting on
            nc.gpsimd.dma_start(
                swp[:].rearrange("p (kt c) -> p kt c", kt=4),
                wp.rearrange("(kt p) c -> p kt c", p=128))

            # ---------------- output projection units ----------------
            # filler mode: psum->sbuf cast on DVE and out-DMA dispatched
            # from the idle Sync queue, so ACT keeps its full bandwidth for
            # the exp chain that paces the late attention windows.
            # tail mode (exp done): alternate ACT/DVE casts and sync/scalar
            # DMA queues for minimum critical-path latency.
            def make_proj_units(i4, attn, tail_mode=False, copy_act=False):
                units = []

                def proj_unit(sub):
                    def emit():
                        ps = psA.tile([128, 1024], F32, tag="mm2")
                        for fg in range(2):
                            for ct in range(4):
                                nc.tensor.matmul(
                                    ps[:, fg * 512:fg * 512 + 512],
                                    attn[:, ct * 512 + sub * 128:
                                         ct * 512 + sub * 128 + 128],
                                    swp[:, ct * C + fg * 512:
                                        ct * C + fg * 512 + 512],
                                    start=(ct == 0), stop=(ct == 3))
                        ot = op.tile([128, 1024], F16, tag="out")
                        if copy_act or (tail_mode and sub % 2):
                            nc.scalar.copy(ot[:], ps[:])
                        else:
                            nc.vector.tensor_copy(ot[:], ps[:])
                        dq = (nc.scalar if (tail_mode and sub % 2 == 0)
                              else nc.sync)
                        dq.dma_start(
                            out[i4 * 512 + sub * 128:
                                i4 * 512 + sub * 128 + 128, :],
                            ot[:])
                    return emit

                for sub in range(4):
                    units.append(proj_unit(sub))
                return units

            # -------- attention for one q-group, fillers interleaved --------
            deferred = []  # normalize tails (bc+mul) deferred for dep slack

            def emit_attention(i4, fillers, tail_fillers=(), mid_hook=None):
                entries = plan[i4]
                first_j = entries[0][0]
                last_j = entries[-1][0]
                E = 4 * len(entries)
                F = len(fillers)
                state = {"e": 0, "fi": 0}
                attn = aop.tile([128, 4 * 512], F16, tag="attn")
                for h0 in range(0, HPC, 2):
                    hm = h0 // 2
                    if hm == 2 and mid_hook is not None:
                        # half-window-early x prefetch: the next group's
                        # qkv fillers then never wait on the transfer
                        mid_hook()
                    ppvs = [psB.tile([VW, 512], F32, tag="pv",
                                     name=f"ppv{hh}") for hh in range(2)]

                    def emit_pv(j, l0, et):
                        w = 512 - l0
                        for hh in range(2):
                            nc.tensor.matmul(
                                ppvs[hh][:, l0:512],
                                sv[:, (j * HPC + h0 + hh) * VW:
                                   (j * HPC + h0 + hh) * VW + VW],
                                et[:, hh * 512 + l0 * (1 - hh):
                                   hh * 512 + l0 * (1 - hh) + w],
                                start=(j == first_j), stop=(j == last_j))

                    pending = []
                    we = 0  # entry index within this hm window
                    for (j, lo, hi, mseg) in entries:
                        l0 = lo * 128
                        w = 512 - l0
                        pscr = psA.tile([128, 1024], F32, tag="mm2")
                        for hh in range(2):
                            hp = hh * 64
                            # hh0 at [l0:512] (bank 0), hh1 at [512:512+w]
                            # (bank 1 start) -> exp window is contiguous
                            o = hh * 512 + l0 * (1 - hh)
                            nc.tensor.matmul(
                                pscr[:, o:o + w],
                                sk[hp:hp + 64,
                                   hm * N + j * 128:hm * N + j * 128 + 128],
                                sq[hp:hp + 64, hm * N + i4 * 512 + l0:
                                   hm * N + i4 * 512 + 512],
                                start=True, stop=True)
                        et = ep.tile([128, 1024], F16, tag="exp")
                        nc.scalar.activation(et[:, l0:512 + w],
                                             pscr[:, l0:512 + w], AF.Exp)
                        if mseg is not None:
                            soff, alo, ahi = mseg
                            mw = (ahi - alo) * 128
                            for hh in range(2):
                                o = (alo * 128 if hh == 0
                                     else 512 + (alo - lo) * 128)
                                nc.vector.tensor_mul(
                                    et[:, o:o + mw], et[:, o:o + mw],
                                    smask[:, soff * 128:soff * 128 + mw])
                        pending.append((j, l0, et))
                        state["e"] += 1
                        we += 1
                        if we == 2:
                            # 2 entries of slack: prior window's recips are
                            # done, so these don't head-of-line-block queues
                            while deferred:
                                deferred.pop(0)()
                        ee = state["e"]
                        EE = E
                        due = min(F, F * ee // EE) - min(F, F * (ee - 1) // EE
                                                        if ee else 0)
                        for _ in range(due):
                            fillers[state["fi"]]()
                            state["fi"] += 1
                        if len(pending) > 3:
                            emit_pv(*pending.pop(0))
                    while pending:
                        emit_pv(*pending.pop(0))
                    # normalize: rows 0..63 / row 64 (the ones-column sums).
                    # srow+recip now; the Pool broadcast + DVE multiply are
                    # deferred into the next window so their upstream waits
                    # never block this window's time-critical DVE/Pool work.
                    tails = []
                    for hh in range(2):
                        hp = hh * 64
                        ppv = ppvs[hh]
                        srow = nrm.tile([1, 512], F32, tag="srow")
                        rec = nrm.tile([1, 512], F32, tag="rec")
                        bc = nrm.tile([64, 512], F32, tag="bc")
                        nc.vector.tensor_copy(srow[:], ppv[64:65, :])
                        nc.vector.reciprocal_approx_fast(rec[:], srow[:])
                        # broadcast now (Pool is idle at window end); only
                        # the DVE multiply is deferred, so the next window's
                        # first PV waits on one short DVE op instead of the
                        # full recip->Pool->mul chain
                        nc.gpsimd.partition_broadcast(bc[:], rec[:])
                        tails.append((hp, ppv, bc))

                    def norm_tail(hm=hm, tails=tails):
                        for hp, ppv, bc in tails:
                            nc.vector.tensor_mul(
                                attn[hp:hp + 64, hm * 512:hm * 512 + 512],
                                ppv[0:64, :], bc[:])
                    deferred.append(norm_tail)
                while state["fi"] < F:
                    fillers[state["fi"]]()
                    state["fi"] += 1
                # tail fillers: PE work emitted after the last window's
                # srow/recip so the deferred normalize chain completes in
                # their shadow instead of stalling whatever follows
                for idx, u in enumerate(tail_fillers):
                    u()
                    if idx == 0:
                        while deferred:
                            deferred.pop(0)()
                return attn

            # Fillers rebalanced by attention-phase load: group i4's windows
            # carry 4*(i4+1) exps (~1.2us each on ACT) but only ~3.7*(i4+1)us
            # of score/PV PE work, so early groups are PE-rich and the last
            # group is ACT-bound.  Defer ALL proj work to the last two
            # groups: i4=2 gets proj(0), i4=3 gets proj(1) + proj(2) (their
            # attn tiles are kept alive by aop bufs=4).
            attns = {}
            for i4 in range(QG):
                if i4 == 0:
                    fetch_x(1)
                mid = ((lambda ng=i4 + 2: fetch_x(ng))
                       if i4 + 2 < QG else None)
                fillers = []
                tail = []
                if i4 + 1 < QG:
                    # whole qkv casts on ACT: every group with qkv fillers
                    # (i4=0..2) has exp slack there, and it keeps DVE free
                    # for the mask/normalize ops that gate the PV chains
                    fillers.extend(make_qkv_units(i4 + 1,
                                                  copy_eng=nc.scalar))
                if i4 == 2:
                    fillers.extend(make_proj_units(0, attns[0], copy_act=True))
                if i4 == 3:
                    p1 = make_proj_units(1, attns[1])
                    p2 = make_proj_units(2, attns[2])
                    fillers.extend(p1)
                    fillers.extend(p2[:1])
                    # hold proj(2) units back as tail cover for the final
                    # normalize chain
                    tail = p2[1:]
                attns[i4] = emit_attention(i4, fillers, tail, mid_hook=mid)
            while deferred:
                deferred.pop(0)()
            for u in make_proj_units(QG - 1, attns[QG - 1], tail_mode=True):
                u()
    nc.compile()
    return nc


def _get_program(attn_mask):
    key = attn_mask.tobytes()
    if key not in _CACHE:
        plan, masks_np = _build_plan(attn_mask)
        nc = _build_program(plan, masks_np.shape[1])
        _CACHE[key] = (nc, masks_np)
    return _CACHE[key]


def _make_in_maps(x, attn_mask, W_qkv, W_proj, masks_np):
    w4 = W_qkv.reshape(C, 3, H, D)
    ones = np.ones((128, 128), dtype=np.float16)
    in_maps = []
    for core in range(8):
        b, g = core // G, core % G
        hs = slice(g * HPC, (g + 1) * HPC)
        wq = (w4[:, 0, hs, :] / np.sqrt(D)).reshape(C, DG)
        wk = w4[:, 1, hs, :].reshape(C, DG)
        wv = w4[:, 2, hs, :].reshape(C, DG)
        in_maps.append({
            "xT": np.ascontiguousarray(x[b].T).astype(np.float16),
            "wqkv": np.ascontiguousarray(
                np.concatenate([wq, wk, wv], axis=1)).astype(np.float16),
            "wp": np.ascontiguousarray(
                W_proj[g * DG:(g + 1) * DG, :]).astype(np.float16),
            "masks": masks_np.astype(np.float16),
            "ones": ones,
        })
    return in_maps


def kernel(x, attn_mask, W_qkv, W_proj, b_proj, **run_kwargs):
    from concourse import bass_utils

    x = np.asarray(x, dtype=np.float32)
    attn_mask = np.asarray(attn_mask, dtype=np.float32)
    W_qkv = np.asarray(W_qkv, dtype=np.float32)
    W_proj = np.asarray(W_proj, dtype=np.float32)
    b_proj = np.asarray(b_proj, dtype=np.float32)

    nc, masks_np = _get_program(attn_mask)
    in_maps = _make_in_maps(x, attn_mask, W_qkv, W_proj, masks_np)

    res = bass_utils.run_bass_kernel_spmd(nc, in_maps, core_ids=list(range(8)),
                                          **run_kwargs)
    outp = np.empty((B, N, C), dtype=np.float32)
    for b in range(B):
        outp[b] = (res.results[2 * b]["out"].astype(np.float32)
                   + res.results[2 * b + 1]["out"].astype(np.float32) + b_proj)
    if run_kwargs:
        kernel.last_result = res
    return outp

